# revision 22
# baseline (speedup 1.0000x reference)
"""Causal multi-head attention (16 heads, hd=64) on 8 trn2 NeuronCores.

Sharding: core c -> batch b = c // 4, head-group g = c % 4 (4 heads = 256
columns of Wq/Wk/Wv).  Each core computes its [S, 256] slice of the three
outputs (attn out, K_cache, V_cache); the host gathers slices.

Per-core pipeline (Tile framework), all matmuls in fp32r (full PE rate at
N>=256, ~1e-4 operand rounding):
  - xT [1024, S] is host-transposed x[b]; weights/biases host-sliced.
  - KT/QT [c, q] computed directly (lhsT = W chunk), per-partition bias
    added during the DVE eviction; K_cache = PE-transpose of kt back to
    natural layout, emitted last (off the critical path).
  - Vf natural [s, c] (rank-1 bias matmul) -> V_cache + V_aug tiles
    [k, 65] per head (ones column -> softmax denominator; ones written
    by DVE -- a strided sub-word DMA would RMW-race adjacent columns).
  - scores ST[k, q]: the two heads of a pair run as concurrent
    row-tiled matmuls (K=64, partition offsets 0/64) into one
    [128, 1024] psum tile; diagonal blocks narrowed to the valid q
    range; one exp per k-tile over both heads (ACT, scale=1/8, per-k
    pad bias), fill-0 affine_select on the 128-wide partial triangle.
  - AV: out_unnorm[65, q] += V_aug.T @ PT over k-tiles; PE transpose
    back to [q, 65]; divide by the ones-row sum (reciprocal +
    tensor_scalar_mul); assemble [128, 256] rows, DMA out.
  - Emission order interleaves projections with attention per q-slice so
    attention starts as soon as its k-range is projected.
"""

import numpy as np

P = 128
S = 2048
HIN = 1024
C = 256  # columns per core = 4 heads * 64
HD = 64
NCORES = 8
HC = HIN // P  # 8 contraction chunks
NKT = S // P  # 16 k-tiles
QW = 512  # q-slice width
NQ = S // QW  # 4 q-slices
NPAIR = C // P  # 2 head-pairs per core

_nc_cache = None


def build_nc():
    import concourse.bacc as bacc
    import concourse.mybir as mybir
    from concourse.tile import TileContext
    from concourse.masks import make_identity
    from contextlib import ExitStack

    f32 = mybir.dt.float32
    f32r = mybir.dt.float32r
    bf16 = mybir.dt.bfloat16
    Exp = mybir.ActivationFunctionType.Exp
    Identity = mybir.ActivationFunctionType.Identity
    is_ge = mybir.AluOpType.is_ge

    nc = bacc.Bacc(None, target_bir_lowering=False)

    xt = nc.declare_dram_parameter("xt", [HIN, S], f32r, isOutput=False)
    wq = nc.declare_dram_parameter("wq", [HIN, C], f32r, isOutput=False)
    wk = nc.declare_dram_parameter("wk", [HIN, C], f32r, isOutput=False)
    wv = nc.declare_dram_parameter("wv", [HIN, C], f32r, isOutput=False)
    bqc = nc.declare_dram_parameter("bqc", [P, NPAIR], f32, isOutput=False)
    bkc = nc.declare_dram_parameter("bkc", [P, NPAIR], f32, isOutput=False)
    bv = nc.declare_dram_parameter("bv", [1, C], f32r, isOutput=False)
    padneg = nc.declare_dram_parameter("padneg", [P, NKT], f32, isOutput=False)
    ones = nc.declare_dram_parameter("ones", [P, C], f32r, isOutput=False)
    out = nc.declare_dram_parameter("out", [S, C], f32, isOutput=True)
    kc = nc.declare_dram_parameter("kc", [S, C], f32, isOutput=True)
    vc = nc.declare_dram_parameter("vc", [S, C], f32, isOutput=True)

    with TileContext(nc) as tc, ExitStack() as ctx:
        persist = ctx.enter_context(tc.tile_pool(name="persist", bufs=1))
        xt_sb = persist.tile([P, HC, S], f32r)
        wq_sb = persist.tile([P, HC, C], f32r)
        wk_sb = persist.tile([P, HC, C], f32r)
        wv_sb = persist.tile([P, HC, C], f32r)
        bqc_sb = persist.tile([P, NPAIR], f32)
        bkc_sb = persist.tile([P, NPAIR], f32)
        bv_sb = persist.tile([1, C], f32r)
        pn_sb = persist.tile([P, NKT], f32)
        ones_sb = persist.tile([P, C], f32r)
        ident = persist.tile([P, P], f32)
        qt_bf = persist.tile([P, NPAIR, S], f32r)
        kt_sb = persist.tile([P, NPAIR, S], f32r)
        va_bf = persist.tile([P, NKT, NPAIR, 2 * (HD + 1)], f32r)
        ofin = persist.tile([P, NKT, C], f32)

        # small constants first, then interleave weights (SWDGE/gpsimd
        # queue) with x chunks (HWDGE/sync) so the chunk-j projection
        # matmuls can start as soon as chunk j has arrived
        nc.sync.dma_start(bqc_sb[:], bqc[:])
        nc.sync.dma_start(bkc_sb[:], bkc[:])
        nc.sync.dma_start(bv_sb[:], bv[:])
        nc.sync.dma_start(pn_sb[:], padneg[:])
        nc.sync.dma_start(ones_sb[:], ones[:])
        # weights per chunk on the gpsimd queue, x in quarter-pieces on
        # sync: fine-grained deps let chunk-j matmuls start at arrival
        quarter = S // 4
        for j in range(HC):
            nc.gpsimd.dma_start(wq_sb[:, j, :], wq[j * P : (j + 1) * P, :])
            nc.gpsimd.dma_start(wk_sb[:, j, :], wk[j * P : (j + 1) * P, :])
            nc.gpsimd.dma_start(wv_sb[:, j, :], wv[j * P : (j + 1) * P, :])
            for h in range(4):
                nc.sync.dma_start(
                    xt_sb[:, j, h * quarter : (h + 1) * quarter],
                    xt[j * P : (j + 1) * P, h * quarter : (h + 1) * quarter],
                )
        # ones columns of V_aug (positions 64 and 129).  Written by DVE, not
        # DMA: a strided sub-word DMA write would RMW-race the adjacent
        # DVE-written V columns.
        ones3 = ones_sb[:, : NKT * NPAIR].rearrange("p (a b) -> p a b", a=NKT)
        nc.vector.tensor_copy(
            out=va_bf[:, :, :, HD : HD + 1], in_=ones3[:, :, :, None]
        )
        nc.vector.tensor_copy(
            out=va_bf[:, :, :, 2 * HD + 1 : 2 * HD + 2], in_=ones3[:, :, :, None]
        )
        make_identity(nc, ident[:])

        psum = ctx.enter_context(tc.tile_pool(name="psum", bufs=2, space="PSUM"))
        work = ctx.enter_context(tc.tile_pool(name="work", bufs=3))

        def kt_qt_slice(qi):
            qsl = slice(qi * QW, (qi + 1) * QW)
            for p in range(NPAIR):
                csl = slice(p * P, (p + 1) * P)
                for w_sb, b_sb, dst, dt_out in (
                    (wk_sb, bkc_sb, None, None),
                    (wq_sb, bqc_sb, qt_bf, bf16),
                ):
                    ps = psum.tile([P, QW], f32, tag="proj", bufs=2, name="p_ps")
                    for j in range(HC):
                        nc.tensor.matmul(
                            ps, w_sb[:, j, csl], xt_sb[:, j, qsl],
                            start=(j == 0), stop=(j == HC - 1),
                        )
                    if dst is None:
                        nc.vector.tensor_scalar_add(
                            kt_sb[:, p, qsl], ps, b_sb[:, p : p + 1]
                        )
                    else:
                        nc.vector.tensor_scalar_add(
                            dst[:, p, qsl], ps, b_sb[:, p : p + 1]
                        )

        def v_wave(qi):
            for i in range(4 * qi, 4 * qi + 4):
                ksl = slice(i * P, (i + 1) * P)
                ps = psum.tile([P, QW], f32, tag="proj", bufs=2, name="v_ps")[:, :C]
                for j in range(HC):
                    nc.tensor.matmul(
                        ps, xt_sb[:, j, ksl], wv_sb[:, j, :],
                        start=(j == 0), stop=False,
                    )
                nc.tensor.matmul(
                    ps, ones_sb[:1, :P], bv_sb[:1, :], start=False, stop=True
                )
                sb = work.tile([P, C], f32, tag="projsb", bufs=4, name="v_sb")
                nc.any.tensor_copy(out=sb[:], in_=ps)
                nc.sync.dma_start(vc[ksl, :], sb[:])
                for p in range(NPAIR):
                    nc.vector.tensor_copy(
                        out=va_bf[:, i, p, 0:HD], in_=sb[:, p * P : p * P + HD]
                    )
                    nc.vector.tensor_copy(
                        out=va_bf[:, i, p, HD + 1 : 2 * HD + 1],
                        in_=sb[:, p * P + HD : (p + 1) * P],
                    )

        def attention(qi):
            for p in range(NPAIR):
                av_a = psum.tile([HD + 1, QW], f32, tag="av", bufs=2, name="av_a")
                av_b = psum.tile([HD + 1, QW], f32, tag="av", bufs=2, name="av_b")
                tmax = 4 * qi + 4
                for t in range(tmax):
                    ksl = slice(t * P, (t + 1) * P)
                    d = t - 4 * qi
                    W = QW if d < 0 else QW - d * P
                    q0 = qi * QW + (0 if d < 0 else d * P)
                    st = psum.tile([P, 2 * QW], f32, tag="st", bufs=2, name="st")
                    nc.tensor.matmul(
                        st[:, 0:W], kt_sb[0:HD, p, ksl],
                        qt_bf[0:HD, p, q0 : q0 + W], start=True, stop=True,
                    )
                    nc.tensor.matmul(
                        st[:, QW : QW + W], kt_sb[HD:P, p, ksl],
                        qt_bf[HD:P, p, q0 : q0 + W], start=True, stop=True,
                    )
                    pt = work.tile([P, 2, QW], f32r, tag="pt", bufs=4, name="pt")
                    st3 = st[:].rearrange("p (h w) -> p h w", h=2)[:, :, 0:W]
                    nc.scalar.activation(
                        pt[:, :, 0:W], st3, Exp, bias=pn_sb[:, t : t + 1],
                        scale=0.125,
                    )
                    if d >= 0:
                        nc.gpsimd.affine_select(
                            out=pt[:, :, 0:P], in_=pt[:, :, 0:P],
                            compare_op=is_ge, fill=0.0, base=0,
                            pattern=[[0, 2], [1, P]], channel_multiplier=-1,
                        )
                    nc.tensor.matmul(
                        av_a[:, QW - W :], va_bf[:, t, p, 0 : HD + 1],
                        pt[:, 0, 0:W], start=(t == 0), stop=(t == tmax - 1),
                    )
                    nc.tensor.matmul(
                        av_b[:, QW - W :], va_bf[:, t, p, HD + 1 : 2 * HD + 2],
                        pt[:, 1, 0:W], start=(t == 0), stop=(t == tmax - 1),
                    )
                for h, av in ((0, av_a), (1, av_b)):
                    osb = work.tile([HD + 1, QW], f32, tag="osb", bufs=3, name="osb")
                    nc.vector.tensor_copy(out=osb[:], in_=av)
                    for sub in range(4):
                        tr = psum.tile(
                            [P, P], f32, tag="av", bufs=2, name="otr"
                        )[:, : HD + 1]
                        nc.tensor.transpose(
                            tr, osb[:, sub * P : (sub + 1) * P],
                            ident[: HD + 1, : HD + 1],
                        )
                        rcp = work.tile([P, 1], f32, tag="rcp", bufs=2, name="rcp")
                        nc.vector.reciprocal(rcp[:], tr[:, HD : HD + 1])
                        i = 4 * qi + sub
                        col = p * P + h * HD
                        nc.vector.tensor_scalar_mul(
                            ofin[:, i, col : col + HD], tr[:, 0:HD], rcp[:]
                        )
            for sub in range(4):
                i = 4 * qi + sub
                nc.sync.dma_start(out[i * P : (i + 1) * P, :], ofin[:, i, :])

        def kc_tiles(qi):
            for i in range(4 * qi, 4 * qi + 4):
                ksl = slice(i * P, (i + 1) * P)
                sb = work.tile([P, C], f32, tag="projsb", bufs=4, name="kc_sb")
                for p in range(NPAIR):
                    tr = psum.tile([P, P], f32, tag="proj", bufs=2, name="kc_tr")
                    nc.tensor.transpose(
                        tr, kt_sb[:, p, ksl].bitcast(f32), ident[:]
                    )
                    nc.vector.tensor_copy(out=sb[:, p * P : (p + 1) * P], in_=tr)
                nc.sync.dma_start(kc[ksl, :], sb[:])

        # interleaved emission: project a q/k-slice, then run the attention
        # that only needs what's already projected
        for qi in range(NQ):
            kt_qt_slice(qi)
            v_wave(qi)
            attention(qi)
        # K_cache transposes last -- off the attention critical path
        for qi in range(NQ):
            kc_tiles(qi)

    nc.finalize()
    return nc


def get_nc():
    global _nc_cache
    if _nc_cache is None:
        _nc_cache = build_nc()
    return _nc_cache


def make_in_maps(x, pad_mask, Wq, bq, Wk, bk, Wv, bv):
    x = np.asarray(x, np.float32)
    pad_mask = np.asarray(pad_mask, np.float32)
    Wq = np.asarray(Wq, np.float32)
    bq = np.asarray(bq, np.float32)
    Wk = np.asarray(Wk, np.float32)
    bk = np.asarray(bk, np.float32)
    Wv = np.asarray(Wv, np.float32)
    bv = np.asarray(bv, np.float32)
    in_maps = []
    for c in range(NCORES):
        b, g = divmod(c, 4)
        cols = slice(g * C, (g + 1) * C)
        xt = np.ascontiguousarray(x[b].T)  # [HIN, S]
        pn = ((pad_mask[b] - 1.0) * 1e6).reshape(NKT, P).T.copy()  # [P, NKT]
        in_maps.append(
            dict(
                xt=xt,
                ones=np.ones((P, C), np.float32),
                wq=np.ascontiguousarray(Wq[:, cols]),
                wk=np.ascontiguousarray(Wk[:, cols]),
                wv=np.ascontiguousarray(Wv[:, cols]),
                bqc=np.ascontiguousarray(bq[cols].reshape(NPAIR, P).T),
                bkc=np.ascontiguousarray(bk[cols].reshape(NPAIR, P).T),
                bv=np.ascontiguousarray(bv[cols].reshape(1, C)),
                padneg=pn,
            )
        )
    return in_maps


def gather(results):
    B = 2
    out = np.empty((B, S, HIN), np.float32)
    kcache = np.empty((B, S, HIN), np.float32)
    vcache = np.empty((B, S, HIN), np.float32)
    for c in range(NCORES):
        b, g = divmod(c, 4)
        cols = slice(g * C, (g + 1) * C)
        out[b, :, cols] = results[c]["out"]
        kcache[b, :, cols] = results[c]["kc"]
        vcache[b, :, cols] = results[c]["vc"]
    return out, kcache, vcache


def kernel(x, pad_mask, Wq, bq, Wk, bk, Wv, bv):
    from concourse.bass_utils import run_bass_kernel_spmd

    nc = get_nc()
    in_maps = make_in_maps(x, pad_mask, Wq, bq, Wk, bk, Wv, bv)
    res = run_bass_kernel_spmd(nc, in_maps, list(range(NCORES)))
    return gather(res.results)


# revision 23
# speedup vs baseline: 1.0248x; 1.0248x over previous
"""Causal multi-head attention (16 heads, hd=64) on 8 trn2 NeuronCores.

Sharding: core c -> batch b = c // 4, head-group g = c % 4 (4 heads = 256
columns of Wq/Wk/Wv).  Each core computes its [S, 256] slice of the three
outputs (attn out, K_cache, V_cache); the host gathers slices.

Per-core pipeline (Tile framework), all matmuls in fp32r (full PE rate at
N>=256, ~1e-4 operand rounding):
  - xT [1024, S] is host-transposed x[b]; weights/biases host-sliced.
  - KT/QT [c, q] computed directly (lhsT = W chunk), per-partition bias
    added during the DVE eviction; K_cache = PE-transpose of kt back to
    natural layout, emitted last (off the critical path).
  - Vf natural [s, c] (rank-1 bias matmul) -> V_cache + V_aug tiles
    [k, 65] per head (ones column -> softmax denominator; ones written
    by DVE -- a strided sub-word DMA would RMW-race adjacent columns).
  - scores ST[k, q]: the two heads of a pair run as concurrent
    row-tiled matmuls (K=64, partition offsets 0/64) into one
    [128, 1024] psum tile; diagonal blocks narrowed to the valid q
    range; one exp per k-tile over both heads (ACT, scale=1/8, per-k
    pad bias), fill-0 affine_select on the 128-wide partial triangle.
  - AV: out_unnorm[65, q] += V_aug.T @ PT over k-tiles; PE transpose
    back to [q, 65]; divide by the ones-row sum (reciprocal +
    tensor_scalar_mul); assemble [128, 256] rows, DMA out.
  - Emission order interleaves projections with attention per q-slice so
    attention starts as soon as its k-range is projected.
"""

import numpy as np

P = 128
S = 2048
HIN = 1024
C = 256  # columns per core = 4 heads * 64
HD = 64
NCORES = 8
HC = HIN // P  # 8 contraction chunks
NKT = S // P  # 16 k-tiles
QW = 512  # q-slice width
NQ = S // QW  # 4 q-slices
NPAIR = C // P  # 2 head-pairs per core

_nc_cache = None


def build_nc():
    import concourse.bacc as bacc
    import concourse.mybir as mybir
    from concourse.tile import TileContext
    from concourse.masks import make_identity
    from contextlib import ExitStack

    f32 = mybir.dt.float32
    f32r = mybir.dt.float32r
    bf16 = mybir.dt.bfloat16
    Exp = mybir.ActivationFunctionType.Exp
    Identity = mybir.ActivationFunctionType.Identity
    is_ge = mybir.AluOpType.is_ge

    nc = bacc.Bacc(None, target_bir_lowering=False)

    xt = nc.declare_dram_parameter("xt", [HIN, S], f32r, isOutput=False)
    wq = nc.declare_dram_parameter("wq", [HIN, C], f32r, isOutput=False)
    wk = nc.declare_dram_parameter("wk", [HIN, C], f32r, isOutput=False)
    wv = nc.declare_dram_parameter("wv", [HIN, C], f32r, isOutput=False)
    bqc = nc.declare_dram_parameter("bqc", [P, NPAIR], f32, isOutput=False)
    bkc = nc.declare_dram_parameter("bkc", [P, NPAIR], f32, isOutput=False)
    bv = nc.declare_dram_parameter("bv", [1, C], f32r, isOutput=False)
    padneg = nc.declare_dram_parameter("padneg", [P, NKT], f32, isOutput=False)
    ones = nc.declare_dram_parameter("ones", [P, C], f32r, isOutput=False)
    out = nc.declare_dram_parameter("out", [S, C], f32, isOutput=True)
    kct = nc.declare_dram_parameter("kct", [C, S], f32, isOutput=True)
    vc = nc.declare_dram_parameter("vc", [S, C], f32, isOutput=True)

    with TileContext(nc) as tc, ExitStack() as ctx:
        persist = ctx.enter_context(tc.tile_pool(name="persist", bufs=1))
        xt_sb = persist.tile([P, HC, S], f32r)
        wq_sb = persist.tile([P, HC, C], f32r)
        wk_sb = persist.tile([P, HC, C], f32r)
        wv_sb = persist.tile([P, HC, C], f32r)
        bqc_sb = persist.tile([P, NPAIR], f32)
        bkc_sb = persist.tile([P, NPAIR], f32)
        bv_sb = persist.tile([1, C], f32r)
        pn_sb = persist.tile([P, NKT], f32)
        ones_sb = persist.tile([P, C], f32r)
        ident = persist.tile([P, P], f32)
        qt_bf = persist.tile([P, NPAIR, S], f32r)
        kt_sb = persist.tile([P, NPAIR, S], f32r)
        va_bf = persist.tile([P, NKT, NPAIR, 2 * (HD + 1)], f32r)
        ofin = persist.tile([P, NKT, C], f32)

        # small constants first, then interleave weights (SWDGE/gpsimd
        # queue) with x chunks (HWDGE/sync) so the chunk-j projection
        # matmuls can start as soon as chunk j has arrived
        nc.sync.dma_start(bqc_sb[:], bqc[:])
        nc.sync.dma_start(bkc_sb[:], bkc[:])
        nc.sync.dma_start(bv_sb[:], bv[:])
        nc.sync.dma_start(pn_sb[:], padneg[:])
        nc.sync.dma_start(ones_sb[:], ones[:])
        # weights per chunk on the gpsimd queue, x in quarter-pieces on
        # sync: fine-grained deps let chunk-j matmuls start at arrival
        quarter = S // 4
        for j in range(HC):
            nc.gpsimd.dma_start(wq_sb[:, j, :], wq[j * P : (j + 1) * P, :])
            nc.gpsimd.dma_start(wk_sb[:, j, :], wk[j * P : (j + 1) * P, :])
            nc.gpsimd.dma_start(wv_sb[:, j, :], wv[j * P : (j + 1) * P, :])
            for h in range(4):
                nc.sync.dma_start(
                    xt_sb[:, j, h * quarter : (h + 1) * quarter],
                    xt[j * P : (j + 1) * P, h * quarter : (h + 1) * quarter],
                )
        # ones columns of V_aug (positions 64 and 129).  Written by DVE, not
        # DMA: a strided sub-word DMA write would RMW-race the adjacent
        # DVE-written V columns.
        ones3 = ones_sb[:, : NKT * NPAIR].rearrange("p (a b) -> p a b", a=NKT)
        nc.vector.tensor_copy(
            out=va_bf[:, :, :, HD : HD + 1], in_=ones3[:, :, :, None]
        )
        nc.vector.tensor_copy(
            out=va_bf[:, :, :, 2 * HD + 1 : 2 * HD + 2], in_=ones3[:, :, :, None]
        )
        make_identity(nc, ident[:])

        psum = ctx.enter_context(tc.tile_pool(name="psum", bufs=2, space="PSUM"))
        work = ctx.enter_context(tc.tile_pool(name="work", bufs=3))

        def kt_qt_slice(qi):
            qsl = slice(qi * QW, (qi + 1) * QW)
            for p in range(NPAIR):
                csl = slice(p * P, (p + 1) * P)
                for w_sb, b_sb, dst, dt_out in (
                    (wk_sb, bkc_sb, None, None),
                    (wq_sb, bqc_sb, qt_bf, bf16),
                ):
                    ps = psum.tile([P, QW], f32, tag="proj", bufs=2, name="p_ps")
                    for j in range(HC):
                        nc.tensor.matmul(
                            ps, w_sb[:, j, csl], xt_sb[:, j, qsl],
                            start=(j == 0), stop=(j == HC - 1),
                        )
                    if dst is None:
                        nc.vector.tensor_scalar_add(
                            kt_sb[:, p, qsl], ps, b_sb[:, p : p + 1]
                        )
                    else:
                        nc.vector.tensor_scalar_add(
                            dst[:, p, qsl], ps, b_sb[:, p : p + 1]
                        )

        def v_wave(qi):
            for i in range(4 * qi, 4 * qi + 4):
                ksl = slice(i * P, (i + 1) * P)
                ps = psum.tile([P, QW], f32, tag="proj", bufs=2, name="v_ps")[:, :C]
                for j in range(HC):
                    nc.tensor.matmul(
                        ps, xt_sb[:, j, ksl], wv_sb[:, j, :],
                        start=(j == 0), stop=False,
                    )
                nc.tensor.matmul(
                    ps, ones_sb[:1, :P], bv_sb[:1, :], start=False, stop=True
                )
                sb = work.tile([P, C], f32, tag="projsb", bufs=4, name="v_sb")
                nc.any.tensor_copy(out=sb[:], in_=ps)
                nc.sync.dma_start(vc[ksl, :], sb[:])
                for p in range(NPAIR):
                    nc.vector.tensor_copy(
                        out=va_bf[:, i, p, 0:HD], in_=sb[:, p * P : p * P + HD]
                    )
                    nc.vector.tensor_copy(
                        out=va_bf[:, i, p, HD + 1 : 2 * HD + 1],
                        in_=sb[:, p * P + HD : (p + 1) * P],
                    )

        def attention(qi):
            for p in range(NPAIR):
                av_a = psum.tile([HD + 1, QW], f32, tag="av", bufs=2, name="av_a")
                av_b = psum.tile([HD + 1, QW], f32, tag="av", bufs=2, name="av_b")
                tmax = 4 * qi + 4
                for t in range(tmax):
                    ksl = slice(t * P, (t + 1) * P)
                    d = t - 4 * qi
                    W = QW if d < 0 else QW - d * P
                    q0 = qi * QW + (0 if d < 0 else d * P)
                    st = psum.tile([P, 2 * QW], f32, tag="st", bufs=2, name="st")
                    nc.tensor.matmul(
                        st[:, 0:W], kt_sb[0:HD, p, ksl],
                        qt_bf[0:HD, p, q0 : q0 + W], start=True, stop=True,
                    )
                    nc.tensor.matmul(
                        st[:, QW : QW + W], kt_sb[HD:P, p, ksl],
                        qt_bf[HD:P, p, q0 : q0 + W], start=True, stop=True,
                    )
                    pt = work.tile([P, 2, QW], f32r, tag="pt", bufs=4, name="pt")
                    st3 = st[:].rearrange("p (h w) -> p h w", h=2)[:, :, 0:W]
                    nc.scalar.activation(
                        pt[:, :, 0:W], st3, Exp, bias=pn_sb[:, t : t + 1],
                        scale=0.125,
                    )
                    if d >= 0:
                        nc.gpsimd.affine_select(
                            out=pt[:, :, 0:P], in_=pt[:, :, 0:P],
                            compare_op=is_ge, fill=0.0, base=0,
                            pattern=[[0, 2], [1, P]], channel_multiplier=-1,
                        )
                    nc.tensor.matmul(
                        av_a[:, QW - W :], va_bf[:, t, p, 0 : HD + 1],
                        pt[:, 0, 0:W], start=(t == 0), stop=(t == tmax - 1),
                    )
                    nc.tensor.matmul(
                        av_b[:, QW - W :], va_bf[:, t, p, HD + 1 : 2 * HD + 2],
                        pt[:, 1, 0:W], start=(t == 0), stop=(t == tmax - 1),
                    )
                for h, av in ((0, av_a), (1, av_b)):
                    osb = work.tile([HD + 1, QW], f32, tag="osb", bufs=3, name="osb")
                    nc.vector.tensor_copy(out=osb[:], in_=av)
                    for sub in range(4):
                        tr = psum.tile(
                            [P, P], f32, tag="av", bufs=2, name="otr"
                        )[:, : HD + 1]
                        nc.tensor.transpose(
                            tr, osb[:, sub * P : (sub + 1) * P],
                            ident[: HD + 1, : HD + 1],
                        )
                        rcp = work.tile([P, 1], f32, tag="rcp", bufs=2, name="rcp")
                        nc.vector.reciprocal(rcp[:], tr[:, HD : HD + 1])
                        i = 4 * qi + sub
                        col = p * P + h * HD
                        nc.vector.tensor_scalar_mul(
                            ofin[:, i, col : col + HD], tr[:, 0:HD], rcp[:]
                        )
            for sub in range(4):
                i = 4 * qi + sub
                nc.sync.dma_start(out[i * P : (i + 1) * P, :], ofin[:, i, :])

        # interleaved emission: project a q/k-slice, then run the attention
        # that only needs what's already projected
        for qi in range(NQ):
            kt_qt_slice(qi)
            v_wave(qi)
            attention(qi)
            # K_cache leaves the chip in kt's [c, s] layout (contiguous
            # DMA); the host transposes it during the gather
            for p in range(NPAIR):
                nc.sync.dma_start(
                    kct[p * P : (p + 1) * P, qi * QW : (qi + 1) * QW],
                    kt_sb[:, p, qi * QW : (qi + 1) * QW].bitcast(f32),
                )

    nc.finalize()
    return nc


def get_nc():
    global _nc_cache
    if _nc_cache is None:
        _nc_cache = build_nc()
    return _nc_cache


def make_in_maps(x, pad_mask, Wq, bq, Wk, bk, Wv, bv):
    x = np.asarray(x, np.float32)
    pad_mask = np.asarray(pad_mask, np.float32)
    Wq = np.asarray(Wq, np.float32)
    bq = np.asarray(bq, np.float32)
    Wk = np.asarray(Wk, np.float32)
    bk = np.asarray(bk, np.float32)
    Wv = np.asarray(Wv, np.float32)
    bv = np.asarray(bv, np.float32)
    in_maps = []
    for c in range(NCORES):
        b, g = divmod(c, 4)
        cols = slice(g * C, (g + 1) * C)
        xt = np.ascontiguousarray(x[b].T)  # [HIN, S]
        pn = ((pad_mask[b] - 1.0) * 1e6).reshape(NKT, P).T.copy()  # [P, NKT]
        in_maps.append(
            dict(
                xt=xt,
                ones=np.ones((P, C), np.float32),
                wq=np.ascontiguousarray(Wq[:, cols]),
                wk=np.ascontiguousarray(Wk[:, cols]),
                wv=np.ascontiguousarray(Wv[:, cols]),
                bqc=np.ascontiguousarray(bq[cols].reshape(NPAIR, P).T),
                bkc=np.ascontiguousarray(bk[cols].reshape(NPAIR, P).T),
                bv=np.ascontiguousarray(bv[cols].reshape(1, C)),
                padneg=pn,
            )
        )
    return in_maps


def gather(results):
    B = 2
    out = np.empty((B, S, HIN), np.float32)
    kcache = np.empty((B, S, HIN), np.float32)
    vcache = np.empty((B, S, HIN), np.float32)
    for c in range(NCORES):
        b, g = divmod(c, 4)
        cols = slice(g * C, (g + 1) * C)
        out[b, :, cols] = results[c]["out"]
        kcache[b, :, cols] = results[c]["kct"].T
        vcache[b, :, cols] = results[c]["vc"]
    return out, kcache, vcache


def kernel(x, pad_mask, Wq, bq, Wk, bk, Wv, bv):
    from concourse.bass_utils import run_bass_kernel_spmd

    nc = get_nc()
    in_maps = make_in_maps(x, pad_mask, Wq, bq, Wk, bk, Wv, bv)
    res = run_bass_kernel_spmd(nc, in_maps, list(range(NCORES)))
    return gather(res.results)
